# revision 4
# baseline (speedup 1.0000x reference)
"""Bass/Trainium2 kernel for nn_BBoxDetectionLoss (YOLO-style bbox detection loss).

Strategy (pure data parallel over 8 NeuronCores, 4 images per core):
  The loss decomposes as
    noobj = 0.5 * (sum_all softplus(obj_pred) - sum_resp softplus(obj_pred)) / n_neg
    obj   =        sum_resp softplus(-obj_pred) / n_pos
    coord = 5 *    sum_resp |bbox_pred - target|^2 / n_pos
  where "resp" is at most 24 cells per image (one per gt box, deduped last-wins).
  Dense work is a softplus-sum over the obj channel (9 MB/core HBM-bound load,
  split into descending-size chunks across both HWDGE queues so the stream
  saturates early and the last-chunk activation tail is short).  The box-target
  stage runs on 4 partitions concurrently with the dense stream; its 4 partial
  sums go out in an early AllGather (which doubles as collective warmup), and a
  second 4-byte AllGather ships the dense sum at the end.  Each core then sums
  the 8 rows with a tiny matmul and applies the final normalization.
"""

import math
import sys

import numpy as np

for _p in ("/opt/trn_rl_repo",):
    if _p not in sys.path:
        sys.path.insert(0, _p)

import concourse.bass as bass
import concourse.tile as tile
from concourse import bacc, mybir
from concourse.bass_utils import run_bass_kernel_spmd

F32 = mybir.dt.float32
I32 = mybir.dt.int32

N_CORES = 8
B, H, W, A, C = 32, 112, 112, 9, 5
NBOX = 24
BL = B // N_CORES                     # images per core = 4
CELLS_L = BL * H * W * A              # 451584 cells per core
ELEMS_L = CELLS_L * C                 # 2257920 f32 per core
P = 128
FPL = ELEMS_L // P                    # 17640 elements per partition
CELLS_PP = CELLS_L // P               # 3528 cells per partition
TOT_CELLS = B * H * W * A             # 3612672 (for n_neg)

# Dense chunking: descending sizes (cells per partition), alternating between
# the two HWDGE queues (sync=SP, scalar=ACT) so both descriptor streams run.
CHUNK_CELLS = [882, 882, 588, 588, 294, 294]
assert sum(CHUNK_CELLS) == CELLS_PP
NCHUNK = len(CHUNK_CELLS)

LAMBDA_COORD = 5.0
LAMBDA_NOOBJ = 0.5

# ---- host-side constants ---------------------------------------------------


def _anchors():
    a = []
    for s in (32, 64, 128):
        for r in (0.5, 1.0, 2.0):
            a.append(
                (
                    np.float32(s * math.sqrt(r) / 224.0),
                    np.float32(s / math.sqrt(r) / 224.0),
                )
            )
    return np.array(a, np.float32)  # [9, 2]


# const tensor layout, [4, KCONST] f32:
#   [0:216)     AWB    anchor w, broadcast per (box i, anchor a), a inner
#   [216:432)   AHB    anchor h
#   [432:648)   AWAHB  aw*ah (f32 product, bit-identical to reference's)
#   [648:864)   IOTA9B float(a)
#   [864:1080)  RAWB   1/aw  (f32)
#   [1080:1296) RAHB   1/ah  (f32)
#   [1296:1872) JGT    pair mask [i, j] (i-major, 24x24): 1.0 iff j > i
#   [1872:1873) BASE   per-partition cell base = p * H*W*A
KCONST = 1880


def _build_const():
    anc = _anchors()
    aw, ah = anc[:, 0], anc[:, 1]
    awah = (aw * ah).astype(np.float32)
    raw = (np.float32(1.0) / aw).astype(np.float32)
    rah = (np.float32(1.0) / ah).astype(np.float32)
    row = np.zeros(KCONST, np.float32)
    row[0:216] = np.tile(aw, NBOX)
    row[216:432] = np.tile(ah, NBOX)
    row[432:648] = np.tile(awah, NBOX)
    row[648:864] = np.tile(np.arange(9, dtype=np.float32), NBOX)
    row[864:1080] = np.tile(raw, NBOX)
    row[1080:1296] = np.tile(rah, NBOX)
    jgt = (np.arange(NBOX)[None, :] > np.arange(NBOX)[:, None]).astype(np.float32)
    row[1296:1872] = jgt.reshape(-1)
    cst = np.broadcast_to(row, (BL, KCONST)).copy()
    cst[:, 1872] = np.arange(BL, dtype=np.float32) * (H * W * A)
    return cst


# ---- bass program ----------------------------------------------------------

MAGIC = 8388608.0  # 2^23: (x + 2^23) - 2^23 rounds x to nearest integer
SPLIT = 4097.0     # 2^12 + 1: Dekker split constant for f32

_DIV_UID = [0]


def _dtile(sm, shape):
    _DIV_UID[0] += 1
    return sm.tile(shape, F32, name=f"dv{_DIV_UID[0]}", tag=f"dv{_DIV_UID[0]}")


def _two_prod_err(nc, sm, q, qh, ql, bh, bl, b_ap, shape):
    """err = q*b - fl(q*b) exactly (Dekker); returns (p, err) tiles."""
    p = _dtile(sm, shape)
    nc.vector.tensor_tensor(out=p[:], in0=q[:], in1=b_ap, op=mybir.AluOpType.mult)
    e = _dtile(sm, shape)
    t = _dtile(sm, shape)
    nc.vector.tensor_mul(e[:], qh[:], bh[:])
    nc.vector.tensor_sub(e[:], e[:], p[:])
    nc.vector.tensor_mul(t[:], qh[:], bl[:])
    nc.vector.tensor_add(e[:], e[:], t[:])
    nc.vector.tensor_mul(t[:], ql[:], bh[:])
    nc.vector.tensor_add(e[:], e[:], t[:])
    nc.vector.tensor_mul(t[:], ql[:], bl[:])
    nc.vector.tensor_add(e[:], e[:], t[:])
    return p, e


def _dekker_split(nc, sm, x_ap, shape):
    """x = xh + xl with xh having <=12 mantissa bits; exact products follow."""
    c = _dtile(sm, shape)
    nc.vector.tensor_scalar_mul(c[:], x_ap, SPLIT)
    u = _dtile(sm, shape)
    nc.vector.tensor_tensor(out=u[:], in0=c[:], in1=x_ap, op=mybir.AluOpType.subtract)
    xh = _dtile(sm, shape)
    nc.vector.tensor_sub(xh[:], c[:], u[:])
    xl = _dtile(sm, shape)
    nc.vector.tensor_tensor(out=xl[:], in0=x_ap, in1=xh[:], op=mybir.AluOpType.subtract)
    return xh, xl


def _exact_div(nc, sm, a_ap, b_ap, shape):
    """q = RN(a/b) bit-exact (positive a, normal b), matching IEEE f32 divide.

    DVE reciprocal is correctly rounded (verified on HW), so q0 = fl(a*RN(1/b))
    is within ~1 ulp of a/b.  The residual r = a - q0*b is computed exactly via
    Dekker TwoProd (no FMA needed); the Newton correction c = r*rec then has
    ~1e-7-ulp error, and the final f32 add q = fl(q0 + c) performs the correct
    rounding itself.  Verified bit-exact vs numpy f32 divide on 10M samples.
    """
    rec = _dtile(sm, shape)
    nc.vector.reciprocal(rec[:], b_ap)
    q0 = _dtile(sm, shape)
    nc.vector.tensor_tensor(out=q0[:], in0=a_ap, in1=rec[:], op=mybir.AluOpType.mult)

    bh, bl = _dekker_split(nc, sm, b_ap, shape)
    qh, ql = _dekker_split(nc, sm, q0[:], shape)
    p, e = _two_prod_err(nc, sm, q0, qh, ql, bh, bl, b_ap, shape)
    r = _dtile(sm, shape)
    nc.vector.tensor_tensor(out=r[:], in0=a_ap, in1=p[:], op=mybir.AluOpType.subtract)
    nc.vector.tensor_sub(r[:], r[:], e[:])
    nc.vector.tensor_mul(r[:], r[:], rec[:])
    q = _dtile(sm, shape)
    nc.vector.tensor_add(q[:], q0[:], r[:])
    return q


# Force exp and ln onto the single combined ACT table set: strip them from
# every other set (indices preserved; act_func_set_id is positional) so
# Bacc's table-load pass emits one ACT_TABLE_LOAD instead of ping-ponging
# between exp_and_others and natural_log on every chunk (~1.3us per load).
def _patch_act_tables():
    import functools

    import concourse.bacc as _bacc
    import concourse.hw_specs as _hs

    orig = _hs.get_activation_tables

    @functools.cache
    def patched(arch):
        t = {k: set(v) for k, v in orig(arch).items()}
        keep = "natural_log_exp_and_others"
        strip = {mybir.ActivationFunctionType.Exp, mybir.ActivationFunctionType.Ln}
        if keep in t and strip <= t[keep]:
            for k in t:
                if k != keep:
                    t[k] = t[k] - strip
        return t

    _bacc.get_activation_tables = patched


_patch_act_tables()


def _build_nc():
    nc = bacc.Bacc(
        "TRN2", target_bir_lowering=False, debug=False, num_devices=N_CORES
    )

    pred = nc.dram_tensor("pred", [ELEMS_L], F32, kind="ExternalInput")
    bbt = nc.dram_tensor("bb", [BL, NBOX * 4], F32, kind="ExternalInput")
    cstt = nc.dram_tensor("cst", [BL, KCONST], F32, kind="ExternalInput")
    outt = nc.dram_tensor("out", [5], F32, kind="ExternalOutput")

    predv = pred[:].rearrange("(p f) -> p f", p=P)          # [128, 17640]
    gatherv = pred[:].rearrange("(n c) -> n c", c=C)        # [451584, 5]

    with tile.TileContext(nc) as tc:
        with (
            tc.tile_pool(name="big", bufs=1) as big,
            tc.tile_pool(name="small", bufs=1) as sm,
            tc.tile_pool(name="psum", bufs=1, space="PSUM") as pp,
            tc.tile_pool(name="dram", bufs=1, space="DRAM") as dp,
        ):
            # ---------------- stage B1: dense chunk loads, issued first -----
            # Alternate queues: even chunks on sync (qSPDynamicHW), odd on
            # scalar (qActDynamicHW) so both HW descriptor generators stream.
            chunks = []
            col = 0
            for i, cc in enumerate(CHUNK_CELLS):
                ch = big.tile([P, cc * C], F32, name=f"chunk{i}", tag=f"chunk{i}")
                eng = nc.sync if i % 2 == 0 else nc.scalar
                eng.dma_start(out=ch[:], in_=predv[:, col : col + cc * C])
                chunks.append(ch)
                col += cc * C

            # small input loads on the software DGE (keeps HWDGE queues free)
            bb = sm.tile([BL, NBOX * 4], F32)
            nc.gpsimd.dma_start(out=bb[:], in_=bbt[:])
            cst = sm.tile([BL, KCONST], F32)
            nc.gpsimd.dma_start(out=cst[:], in_=cstt[:])

            ones = sm.tile([P, 1], F32)
            nc.gpsimd.memset(ones[:], 1.0)
            rhs4 = sm.tile([P, 4], F32)
            nc.gpsimd.memset(rhs4[:], 0.0)
            res = sm.tile([1, 8], F32)
            nc.gpsimd.memset(res[:], 0.0)

            # ---------------- stage A: box targets (4 partitions) -----------
            bb3 = bb[:].rearrange("p (i c) -> p i c", c=4)
            cxv, cyv, wv, hv = (bb3[:, :, k] for k in range(4))
            AWB = cst[:, 0:216]
            AHB = cst[:, 216:432]
            AWAHB = cst[:, 432:648]
            IOTA9B = cst[:, 648:864]
            RAWB = cst[:, 864:1080]
            RAHB = cst[:, 1080:1296]
            JGT = cst[:, 1296:1872]
            BASE = cst[:, 1872:1873]

            sx = sm.tile([BL, NBOX], F32)
            sy = sm.tile([BL, NBOX], F32)
            nc.vector.tensor_scalar_mul(sx[:], cxv, float(W))
            nc.vector.tensor_scalar_mul(sy[:], cyv, float(H))
            # floor via 2^23 round-trip (RN) + correction, then clip to [0, W-1]
            gx = sm.tile([BL, NBOX], F32)
            gy = sm.tile([BL, NBOX], F32)
            corr = sm.tile([BL, NBOX], F32)
            for gv, sv, hi in ((gx, sx, W - 1), (gy, sy, H - 1)):
                nc.vector.tensor_scalar(
                    gv[:], sv[:], MAGIC, -MAGIC,
                    op0=mybir.AluOpType.add, op1=mybir.AluOpType.add,
                )
                nc.vector.tensor_tensor(
                    out=corr[:], in0=gv[:], in1=sv[:], op=mybir.AluOpType.is_gt
                )
                nc.vector.tensor_sub(gv[:], gv[:], corr[:])
                nc.vector.tensor_scalar(
                    gv[:], gv[:], float(hi), 0.0,
                    op0=mybir.AluOpType.min, op1=mybir.AluOpType.max,
                )
            tx = sm.tile([BL, NBOX], F32)
            ty = sm.tile([BL, NBOX], F32)
            nc.vector.tensor_sub(tx[:], sx[:], gx[:])
            nc.vector.tensor_sub(ty[:], sy[:], gy[:])

            # validity: any coord nonzero
            vmax = sm.tile([BL, NBOX], F32)
            nc.vector.tensor_reduce(
                vmax[:], bb3, axis=mybir.AxisListType.X,
                op=mybir.AluOpType.max, apply_absolute_value=True,
            )
            valid = sm.tile([BL, NBOX], F32)
            nc.vector.tensor_scalar(
                valid[:], vmax[:], 0.0, None, op0=mybir.AluOpType.is_gt
            )

            # IoU against 9 anchors -> best (first max wins).  The quotient must
            # be bit-exact IEEE f32 division: exact ties between anchors decide
            # argmax, and the reference breaks them by first-index.
            t216a = sm.tile([BL, 216], F32)
            t216b = sm.tile([BL, 216], F32)
            w9 = wv.to_broadcast([BL, NBOX, 9])
            h9 = hv.to_broadcast([BL, NBOX, 9])
            a3 = lambda ap: ap.rearrange("p (i a) -> p i a", a=9)
            nc.vector.tensor_tensor(
                out=a3(t216a[:]), in0=w9, in1=a3(AWB), op=mybir.AluOpType.min
            )
            nc.vector.tensor_tensor(
                out=a3(t216b[:]), in0=h9, in1=a3(AHB), op=mybir.AluOpType.min
            )
            nc.vector.tensor_mul(t216a[:], t216a[:], t216b[:])  # inter
            wh = sm.tile([BL, NBOX], F32)
            nc.vector.tensor_mul(wh[:], wv, hv)
            nc.vector.tensor_tensor(
                out=a3(t216b[:]), in0=wh[:].to_broadcast([BL, NBOX, 9]),
                in1=a3(AWAHB), op=mybir.AluOpType.add,
            )
            nc.vector.tensor_sub(t216b[:], t216b[:], t216a[:])  # union
            nc.vector.tensor_scalar_add(t216b[:], t216b[:], 1e-16)
            iou = _exact_div(nc, sm, t216a[:], t216b[:], [BL, 216])

            ioumax = sm.tile([BL, NBOX], F32)
            nc.vector.tensor_reduce(
                ioumax[:], a3(iou[:]), axis=mybir.AxisListType.X,
                op=mybir.AluOpType.max,
            )
            # val = eq ? a : 9  ->  val = eq * (a - 9) + 9 ; best = min(val)
            nc.vector.tensor_tensor(
                out=a3(t216a[:]), in0=a3(iou[:]),
                in1=ioumax[:].to_broadcast([BL, NBOX, 9]),
                op=mybir.AluOpType.is_equal,
            )
            nc.vector.tensor_scalar_add(t216b[:], IOTA9B, -9.0)
            nc.vector.tensor_mul(t216b[:], t216a[:], t216b[:])
            nc.vector.tensor_scalar_add(t216b[:], t216b[:], 9.0)
            best = sm.tile([BL, NBOX], F32)
            nc.vector.tensor_reduce(
                best[:], a3(t216b[:]), axis=mybir.AxisListType.X,
                op=mybir.AluOpType.min,
            )

            # one-hot select of 1/aw, 1/ah
            nc.vector.tensor_tensor(
                out=a3(t216a[:]), in0=a3(IOTA9B),
                in1=best[:].to_broadcast([BL, NBOX, 9]),
                op=mybir.AluOpType.is_equal,
            )
            rawsel = sm.tile([BL, NBOX], F32)
            rahsel = sm.tile([BL, NBOX], F32)
            nc.vector.tensor_mul(t216b[:], t216a[:], RAWB)
            nc.vector.tensor_reduce(
                rawsel[:], a3(t216b[:]), axis=mybir.AxisListType.X,
                op=mybir.AluOpType.add,
            )
            nc.vector.tensor_mul(t216b[:], t216a[:], RAHB)
            nc.vector.tensor_reduce(
                rahsel[:], a3(t216b[:]), axis=mybir.AxisListType.X,
                op=mybir.AluOpType.add,
            )
            # tw = ln(w/aw + 1e-16), th = ln(h/ah + 1e-16)
            twv = sm.tile([BL, NBOX], F32)
            thv = sm.tile([BL, NBOX], F32)
            nc.vector.tensor_mul(twv[:], wv, rawsel[:])
            nc.vector.tensor_mul(thv[:], hv, rahsel[:])
            nc.vector.tensor_scalar_add(twv[:], twv[:], 1e-16)
            nc.vector.tensor_scalar_add(thv[:], thv[:], 1e-16)
            nc.scalar.activation(twv[:], twv[:], mybir.ActivationFunctionType.Ln)
            nc.scalar.activation(thv[:], thv[:], mybir.ActivationFunctionType.Ln)

            # cell id and flat offsets
            cellf = sm.tile([BL, NBOX], F32)
            nc.vector.tensor_scalar_mul(cellf[:], gy[:], float(W))
            nc.vector.tensor_add(cellf[:], cellf[:], gx[:])
            nc.vector.tensor_scalar_mul(cellf[:], cellf[:], float(A))
            nc.vector.tensor_add(cellf[:], cellf[:], best[:])
            offf = sm.tile([BL, NBOX], F32)
            nc.vector.tensor_scalar(
                offf[:], cellf[:], BASE, None, op0=mybir.AluOpType.add
            )
            offi = sm.tile([BL, NBOX], I32)
            nc.vector.tensor_copy(offi[:], offf[:])

            # dedup: box i dies if a later valid box j hits the same cell
            p3 = lambda ap: ap.rearrange("p (i j) -> p i j", j=NBOX)
            eqp = sm.tile([BL, NBOX * NBOX], F32)
            nc.vector.tensor_tensor(
                out=p3(eqp[:]),
                in0=cellf[:].to_broadcast([BL, NBOX, NBOX]),
                in1=cellf[:][:, None, :].broadcast_to([BL, NBOX, NBOX]),
                op=mybir.AluOpType.is_equal,
            )
            nc.vector.tensor_mul(eqp[:], eqp[:], JGT)
            nc.vector.tensor_tensor(
                out=p3(eqp[:]), in0=p3(eqp[:]),
                in1=valid[:][:, None, :].broadcast_to([BL, NBOX, NBOX]),
                op=mybir.AluOpType.mult,
            )
            dead = sm.tile([BL, NBOX], F32)
            nc.vector.tensor_reduce(
                dead[:], p3(eqp[:]), axis=mybir.AxisListType.X,
                op=mybir.AluOpType.max,
            )
            live = sm.tile([BL, NBOX], F32)
            nc.vector.tensor_mul(live[:], valid[:], dead[:])
            nc.vector.tensor_sub(live[:], valid[:], live[:])

            npos_p = sm.tile([BL, 1], F32)
            nc.vector.tensor_reduce(
                npos_p[:], live[:], axis=mybir.AxisListType.X,
                op=mybir.AluOpType.add,
            )

            # gather responsible predictions: one indirect DMA, 96 rows of 5
            g = sm.tile([BL, NBOX * C], F32)
            # HW indirect DMA consumes ONE offset per partition row, so
            # spread the 96 boxes across 96 partitions for the gather.
            off96 = sm.tile([BL * NBOX, 1], I32)
            nc.sync.dma_start(out=off96[:], in_=offi[:])
            g96 = sm.tile([BL * NBOX, C], F32)
            nc.gpsimd.indirect_dma_start(
                out=g96[:],
                out_offset=None,
                in_=gatherv,
                in_offset=bass.IndirectOffsetOnAxis(ap=off96[:], axis=0),
            )
            nc.sync.dma_start(out=g[:], in_=g96[:])
            g5 = g[:].rearrange("p (i c) -> p i c", c=C)

            # gathered-cell softplus: Exp now, Ln after the dense Exps
            spn = sm.tile([BL, NBOX], F32)
            spp = sm.tile([BL, NBOX], F32)
            nc.scalar.activation(
                spn[:], g5[:, :, 4], mybir.ActivationFunctionType.Exp, scale=-1.0
            )
            nc.scalar.activation(
                spn[:], spn[:], mybir.ActivationFunctionType.Ln, bias=1.0
            )
            nc.scalar.activation(
                spp[:], g5[:, :, 4], mybir.ActivationFunctionType.Exp
            )
            nc.scalar.activation(
                spp[:], spp[:], mybir.ActivationFunctionType.Ln, bias=1.0
            )
            obj_p = sm.tile([BL, 1], F32)
            sub_p = sm.tile([BL, 1], F32)
            spl = sm.tile([BL, NBOX], F32)
            nc.vector.tensor_mul(spl[:], spn[:], live[:])
            nc.vector.tensor_reduce(
                obj_p[:], spl[:], axis=mybir.AxisListType.X,
                op=mybir.AluOpType.add,
            )
            # fold the 0.5 noobj weight into the subtracted term here
            spl2 = sm.tile([BL, NBOX], F32)
            nc.vector.tensor_mul(spl2[:], spp[:], live[:])
            nc.vector.tensor_scalar_mul(spl2[:], spl2[:], LAMBDA_NOOBJ)
            nc.vector.tensor_reduce(
                sub_p[:], spl2[:], axis=mybir.AxisListType.X,
                op=mybir.AluOpType.add,
            )

            # coord = 5 * sum_c (pred_c - t_c)^2, masked by live
            d = sm.tile([BL, NBOX * 4], F32)
            d3 = d[:].rearrange("p (i c) -> p i c", c=4)
            for cidx, tv in enumerate((tx, ty, twv, thv)):
                nc.vector.tensor_tensor(
                    out=d3[:, :, cidx], in0=g5[:, :, cidx], in1=tv[:],
                    op=mybir.AluOpType.subtract,
                )
            nc.vector.tensor_mul(d[:], d[:], d[:])
            cb = sm.tile([BL, NBOX], F32)
            nc.vector.tensor_reduce(
                cb[:], d3, axis=mybir.AxisListType.X, op=mybir.AluOpType.add
            )
            coord_p = sm.tile([BL, 1], F32)
            cbl = sm.tile([BL, NBOX], F32)
            nc.vector.tensor_mul(cbl[:], cb[:], live[:])
            nc.vector.tensor_scalar_mul(cbl[:], cbl[:], LAMBDA_COORD)
            nc.vector.tensor_reduce(
                coord_p[:], cbl[:], axis=mybir.AxisListType.X,
                op=mybir.AluOpType.add,
            )

            # ---------------- AG#1: box partials (early; warms the mesh) ----
            # rhs4 cols: [0.5*sub, obj, 5*coord, npos] on partitions 0..3
            nc.vector.tensor_copy(rhs4[0:BL, 0:1], sub_p[:])
            nc.vector.tensor_copy(rhs4[0:BL, 1:2], obj_p[:])
            nc.vector.tensor_copy(rhs4[0:BL, 2:3], coord_p[:])
            nc.vector.tensor_copy(rhs4[0:BL, 3:4], npos_p[:])
            ps1 = pp.tile([1, 4], F32)
            nc.tensor.matmul(ps1[:], lhsT=ones[:], rhs=rhs4[:], start=True, stop=True)
            bx = sm.tile([1, 4], F32)
            nc.vector.tensor_copy(bx[:], ps1[:])
            ag1_in = dp.tile([1, 4], F32)
            ag1_out = dp.tile([N_CORES, 4], F32, addr_space="Shared")
            nc.sync.dma_start(out=ag1_in[:], in_=bx[:])
            nc.gpsimd.collective_compute(
                "AllGather",
                mybir.AluOpType.bypass,
                replica_groups=[list(range(N_CORES))],
                ins=[ag1_in[:].opt()],
                outs=[ag1_out[:].opt()],
            )

            # ---------------- stage B2: dense softplus over obj channel -----
            # softplus(x) = ln(exp(x) + 1); exp and ln share one ACT table set
            accs = sm.tile([P, NCHUNK], F32)
            for i, ch in enumerate(chunks):
                cc = CHUNK_CELLS[i]
                sp = big.tile([P, cc], F32, name=f"sp{i}", tag=f"sp{i}")
                nc.scalar.activation(
                    sp[:], ch[:, 4::5], mybir.ActivationFunctionType.Exp
                )
                nc.scalar.activation(
                    sp[:], sp[:], mybir.ActivationFunctionType.Ln, bias=1.0,
                    accum_out=accs[:, i : i + 1],
                )

            # ---------------- AG#2: dense partial sum -----------------------
            ps2 = pp.tile([1, NCHUNK], F32)
            nc.tensor.matmul(ps2[:], lhsT=ones[:], rhs=accs[:], start=True, stop=True)
            ds = sm.tile([1, 1], F32)
            nc.vector.tensor_reduce(
                ds[:], ps2[:], axis=mybir.AxisListType.X, op=mybir.AluOpType.add
            )
            ag2_in = dp.tile([1, 1], F32)
            ag2_out = dp.tile([N_CORES, 1], F32, addr_space="Shared")
            nc.sync.dma_start(out=ag2_in[:], in_=ds[:])
            nc.gpsimd.collective_compute(
                "AllGather",
                mybir.AluOpType.bypass,
                replica_groups=[list(range(N_CORES))],
                ins=[ag2_in[:].opt()],
                outs=[ag2_out[:].opt()],
            )

            # ---------------- stage C: combine 8 rows, normalize ------------
            fin = sm.tile([N_CORES, 5], F32)
            nc.sync.dma_start(out=fin[:, 0:4], in_=ag1_out[:])
            nc.sync.dma_start(out=fin[:, 4:5], in_=ag2_out[:])
            ps3 = pp.tile([1, 5], F32)
            nc.tensor.matmul(
                ps3[:], lhsT=ones[0:N_CORES, :], rhs=fin[:], start=True, stop=True
            )
            gsum = sm.tile([1, 5], F32)
            nc.vector.tensor_copy(gsum[:], ps3[:])
            # gsum = [0.5*sub, obj, 5*coord, npos, dense]
            den = sm.tile([1, 2], F32)
            # den[0] = max(npos, 1); den[1] = TOT_CELLS - npos  (n_neg >> 1 always)
            nc.vector.tensor_scalar(
                den[:, 0:1], gsum[:, 3:4], 1.0, None, op0=mybir.AluOpType.max
            )
            nc.vector.tensor_scalar(
                den[:, 1:2], gsum[:, 3:4], -1.0, float(TOT_CELLS),
                op0=mybir.AluOpType.mult, op1=mybir.AluOpType.add,
            )
            rec = sm.tile([1, 2], F32)
            nc.vector.reciprocal(rec[:], den[:])

            # coord = 5*coord_sum / npos
            nc.vector.tensor_tensor(
                out=res[:, 1:2], in0=gsum[:, 2:3], in1=rec[:, 0:1],
                op=mybir.AluOpType.mult,
            )
            # obj = obj_sum / npos
            nc.vector.tensor_tensor(
                out=res[:, 2:3], in0=gsum[:, 1:2], in1=rec[:, 0:1],
                op=mybir.AluOpType.mult,
            )
            # noobj = (0.5*dense - 0.5*sub) / n_neg
            nc.vector.tensor_scalar(
                res[:, 3:4], gsum[:, 4:5], LAMBDA_NOOBJ, None,
                op0=mybir.AluOpType.mult,
            )
            nc.vector.tensor_sub(res[:, 3:4], res[:, 3:4], gsum[:, 0:1])
            nc.vector.tensor_tensor(
                out=res[:, 3:4], in0=res[:, 3:4], in1=rec[:, 1:2],
                op=mybir.AluOpType.mult,
            )
            # total
            nc.vector.tensor_add(res[:, 0:1], res[:, 1:2], res[:, 2:3])
            nc.vector.tensor_add(res[:, 0:1], res[:, 0:1], res[:, 3:4])

            nc.sync.dma_start(out=outt[:], in_=res[0:1, 0:5])

    nc.compile()
    return nc


_NC_CACHE = None


def _get_nc():
    global _NC_CACHE
    if _NC_CACHE is None:
        _NC_CACHE = _build_nc()
    return _NC_CACHE


def kernel_with_results(predictions, bboxes, **run_kwargs):
    predictions = np.ascontiguousarray(predictions, dtype=np.float32)
    bboxes = np.ascontiguousarray(bboxes, dtype=np.float32)
    assert predictions.shape == (B, H, W, A, C)
    assert bboxes.shape == (B, NBOX, 4)

    cst = _build_const()
    in_maps = []
    for c in range(N_CORES):
        shard_p = predictions[c * BL : (c + 1) * BL].reshape(-1)
        shard_b = bboxes[c * BL : (c + 1) * BL].reshape(BL, NBOX * 4)
        in_maps.append({"pred": shard_p, "bb": shard_b, "cst": cst})

    nc = _get_nc()
    res = run_bass_kernel_spmd(nc, in_maps, core_ids=list(range(N_CORES)), **run_kwargs)
    out = np.asarray(res.results[0]["out"], dtype=np.float32).reshape(5)
    return out, res


def kernel(predictions, bboxes):
    out, _ = kernel_with_results(predictions, bboxes)
    return out


# revision 6
# speedup vs baseline: 1.0929x; 1.0929x over previous
"""Bass/Trainium2 kernel for nn_BBoxDetectionLoss (YOLO-style bbox detection loss).

Strategy (pure data parallel over 8 NeuronCores, 4 images per core):
  The loss decomposes as
    noobj = 0.5 * (sum_all softplus(obj_pred) - sum_resp softplus(obj_pred)) / n_neg
    obj   =        sum_resp softplus(-obj_pred) / n_pos
    coord = 5 *    sum_resp |bbox_pred - target|^2 / n_pos
  where "resp" is at most 24 cells per image (one per gt box, deduped last-wins).

  Dense work is a softplus-sum over the obj channel: a 9 MB/core HBM-bound
  stream in descending-size chunks, all on the sync HWDGE queue (chunk data
  would head-of-line-block any later DMA on that queue, so every small DMA
  rides the scalar HWDGE queue instead, ordered by dependency readiness).

  The box-target stage runs one box per partition (96 partitions): short
  chains, cheap [96,9] IoU reciprocal, gather offsets born in gather layout.
  Its 4 partial sums leave in an early AllGather, which doubles as the
  collective warm-up: the first collective of a NEFF pays ~11 us of mesh
  arming that this hides under the dense stream, so the final 4-byte
  AllGather of the dense sum enters the mesh in ~1 us.  Each core sums the
  gathered rows with a tiny matmul; everything that only depends on AG#1
  (n_pos normalization, coord/obj terms) is precomputed before AG#2 lands.
"""

import math
import sys

import numpy as np

for _p in ("/opt/trn_rl_repo",):
    if _p not in sys.path:
        sys.path.insert(0, _p)

import concourse.bass as bass
import concourse.tile as tile
from concourse import bacc, mybir
from concourse.bass_utils import run_bass_kernel_spmd

F32 = mybir.dt.float32
I32 = mybir.dt.int32

N_CORES = 8
B, H, W, A, C = 32, 112, 112, 9, 5
NBOX = 24
BL = B // N_CORES                     # images per core = 4
NB = BL * NBOX                        # boxes per core = 96 (one per partition)
CELLS_L = BL * H * W * A              # 451584 cells per core
ELEMS_L = CELLS_L * C                 # 2257920 f32 per core
P = 128
FPL = ELEMS_L // P                    # 17640 elements per partition
CELLS_PP = CELLS_L // P               # 3528 cells per partition
TOT_CELLS = B * H * W * A             # 3612672 (for n_neg)

# Dense chunking: descending sizes (cells per partition) so the last chunk's
# activation tail after the final DMA byte is short.
CHUNK_CELLS = [882, 882, 588, 588, 294, 294]
assert sum(CHUNK_CELLS) == CELLS_PP
NCHUNK = len(CHUNK_CELLS)

LAMBDA_COORD = 5.0
LAMBDA_NOOBJ = 0.5

# cst96 column layout ([96, K96] f32)
C_AW, C_AH, C_AWAH, C_IOTA, C_IOTAM, C_RAW, C_RAH = (
    0, 9, 18, 27, 36, 45, 54
)
C_BASE = 63
C_EPS = 64
K96 = 65

MAGIC = 8388608.0  # 2^23: (x + 2^23) - 2^23 rounds x to nearest integer


def _anchors():
    a = []
    for s in (32, 64, 128):
        for r in (0.5, 1.0, 2.0):
            a.append(
                (
                    np.float32(s * math.sqrt(r) / 224.0),
                    np.float32(s / math.sqrt(r) / 224.0),
                )
            )
    return np.array(a, np.float32)  # [9, 2]


def _build_const96():
    anc = _anchors()
    aw, ah = anc[:, 0], anc[:, 1]
    row = np.zeros(K96, np.float32)
    row[C_AW:C_AW + 9] = aw
    row[C_AH:C_AH + 9] = ah
    row[C_AWAH:C_AWAH + 9] = (aw * ah).astype(np.float32)
    row[C_IOTA:C_IOTA + 9] = np.arange(9, dtype=np.float32)
    row[C_IOTAM:C_IOTAM + 9] = np.arange(9, dtype=np.float32) - 9.0
    row[C_RAW:C_RAW + 9] = (np.float32(1.0) / aw).astype(np.float32)
    row[C_RAH:C_RAH + 9] = (np.float32(1.0) / ah).astype(np.float32)
    cst = np.broadcast_to(row, (NB, K96)).copy()
    cst[:, C_BASE] = (np.arange(NB) // NBOX).astype(np.float32) * (H * W * A)
    cst[:, C_EPS] = np.float32(1e-16)
    return cst


def _build_jgt():
    # pair mask [i, j] (i-major, 24x24): 1.0 iff j > i, same for all 4 images
    jgt = (np.arange(NBOX)[None, :] > np.arange(NBOX)[:, None]).astype(np.float32)
    return np.broadcast_to(jgt.reshape(-1), (BL, NBOX * NBOX)).copy()


# Force exp and ln onto the single combined ACT table set: strip them from
# every other set (indices preserved; act_func_set_id is positional) so
# Bacc's table-load pass emits one ACT_TABLE_LOAD instead of ping-ponging
# between exp_and_others and natural_log on every chunk (~1.3us per load).
def _patch_act_tables():
    import functools

    import concourse.bacc as _bacc
    import concourse.hw_specs as _hs

    orig = _hs.get_activation_tables

    @functools.cache
    def patched(arch):
        t = {k: set(v) for k, v in orig(arch).items()}
        keep = "natural_log_exp_and_others"
        strip = {mybir.ActivationFunctionType.Exp, mybir.ActivationFunctionType.Ln}
        if keep in t and strip <= t[keep]:
            for k in t:
                if k != keep:
                    t[k] = t[k] - strip
        return t

    _bacc.get_activation_tables = patched


_patch_act_tables()


def _build_nc():
    nc = bacc.Bacc(
        "TRN2", target_bir_lowering=False, debug=False, num_devices=N_CORES
    )

    pred = nc.dram_tensor("pred", [ELEMS_L], F32, kind="ExternalInput")
    bbt = nc.dram_tensor("bb", [NB, 4], F32, kind="ExternalInput")
    cstt = nc.dram_tensor("cst", [NB, K96], F32, kind="ExternalInput")
    jgtt = nc.dram_tensor("jgt", [BL, NBOX * NBOX], F32, kind="ExternalInput")
    outt = nc.dram_tensor("out", [5], F32, kind="ExternalOutput")

    predv = pred[:].rearrange("(p f) -> p f", p=P)          # [128, 17640]
    gatherv = pred[:].rearrange("(n c) -> n c", c=C)        # [451584, 5]

    with tile.TileContext(nc) as tc:
        with (
            tc.tile_pool(name="big", bufs=1) as big,
            tc.tile_pool(name="small", bufs=1) as sm,
            tc.tile_pool(name="psum", bufs=1, space="PSUM") as pp,
            tc.tile_pool(name="dram", bufs=1, space="DRAM") as dp,
        ):
            # ---- dense chunk loads: sync HWDGE queue only, issued first ----
            chunks = []
            col = 0
            for i, cc in enumerate(CHUNK_CELLS):
                ch = big.tile([P, cc * C], F32, name=f"chunk{i}", tag=f"chunk{i}")
                nc.sync.dma_start(out=ch[:], in_=predv[:, col : col + cc * C])
                chunks.append(ch)
                col += cc * C

            # ---- small loads: scalar HWDGE queue (drains immediately) ------
            bb = sm.tile([NB, 4], F32)
            nc.scalar.dma_start(out=bb[:], in_=bbt[:])
            cst = sm.tile([NB, K96], F32)
            nc.scalar.dma_start(out=cst[:], in_=cstt[:])
            jgt = sm.tile([BL, NBOX * NBOX], F32)
            nc.scalar.dma_start(out=jgt[:], in_=jgtt[:])

            ones = sm.tile([P, 1], F32)
            nc.gpsimd.memset(ones[:], 1.0)
            res = sm.tile([1, 8], F32)
            nc.gpsimd.memset(res[:], 0.0)

            AW = cst[:, C_AW:C_AW + 9]
            AH = cst[:, C_AH:C_AH + 9]
            AWAH = cst[:, C_AWAH:C_AWAH + 9]
            IOTA = cst[:, C_IOTA:C_IOTA + 9]
            IOTAM = cst[:, C_IOTAM:C_IOTAM + 9]
            RAW = cst[:, C_RAW:C_RAW + 9]
            RAH = cst[:, C_RAH:C_RAH + 9]
            BASE = cst[:, C_BASE:C_BASE + 1]

            wv = bb[:, 2:3]
            hv = bb[:, 3:4]

            # ---- box stage, one box per partition (96 partitions) ----------
            # grid cell: gxy = clip(floor(cxy * 112), 0, 111)   (W == H == 112)
            sxy = sm.tile([NB, 2], F32)
            nc.vector.tensor_scalar_mul(sxy[:], bb[:, 0:2], float(W))
            gxy = sm.tile([NB, 2], F32)
            nc.vector.tensor_scalar(
                gxy[:], sxy[:], MAGIC, -MAGIC,
                op0=mybir.AluOpType.add, op1=mybir.AluOpType.add,
            )
            corr = sm.tile([NB, 2], F32)
            nc.vector.tensor_tensor(
                out=corr[:], in0=gxy[:], in1=sxy[:], op=mybir.AluOpType.is_gt
            )
            nc.vector.tensor_sub(gxy[:], gxy[:], corr[:])
            nc.vector.tensor_scalar(
                gxy[:], gxy[:], float(W - 1), 0.0,
                op0=mybir.AluOpType.min, op1=mybir.AluOpType.max,
            )

            # IoU against the 9 anchors; ties must break to the first index,
            # which recip+mult preserves (equal inputs give equal outputs).
            w9 = wv.to_broadcast([NB, 1, 9])
            h9 = hv.to_broadcast([NB, 1, 9])
            a3 = lambda ap: ap.rearrange("p (i a) -> p i a", a=9)
            inter = sm.tile([NB, 9], F32)
            uni = sm.tile([NB, 9], F32)
            nc.vector.tensor_tensor(
                out=a3(inter[:]), in0=w9, in1=a3(AW), op=mybir.AluOpType.min
            )
            nc.vector.tensor_tensor(
                out=a3(uni[:]), in0=h9, in1=a3(AH), op=mybir.AluOpType.min
            )
            nc.vector.tensor_mul(inter[:], inter[:], uni[:])
            wh = sm.tile([NB, 1], F32)
            nc.vector.tensor_mul(wh[:], wv, hv)
            nc.vector.tensor_tensor(
                out=a3(uni[:]), in0=wh[:].to_broadcast([NB, 1, 9]),
                in1=a3(AWAH), op=mybir.AluOpType.add,
            )
            nc.vector.tensor_sub(uni[:], uni[:], inter[:])
            nc.vector.tensor_scalar_add(uni[:], uni[:], 1e-16)
            iou = sm.tile([NB, 9], F32)
            nc.vector.reciprocal(iou[:], uni[:])
            nc.vector.tensor_mul(iou[:], iou[:], inter[:])

            ioumax = sm.tile([NB, 1], F32)
            nc.vector.tensor_reduce(
                ioumax[:], a3(iou[:]), axis=mybir.AxisListType.X,
                op=mybir.AluOpType.max,
            )
            # val = eq ? a : 9  ->  val = eq * (a - 9) + 9 ; best = min(val)
            key = sm.tile([NB, 9], F32)
            nc.vector.tensor_tensor(
                out=a3(key[:]), in0=a3(iou[:]),
                in1=ioumax[:].to_broadcast([NB, 1, 9]),
                op=mybir.AluOpType.is_equal,
            )
            nc.vector.tensor_mul(key[:], key[:], IOTAM)
            nc.vector.tensor_scalar_add(key[:], key[:], 9.0)
            best = sm.tile([NB, 1], F32)
            nc.vector.tensor_reduce(
                best[:], a3(key[:]), axis=mybir.AxisListType.X,
                op=mybir.AluOpType.min,
            )

            # cell id (into cv col 0, next to validity in col 1) and offsets
            cv = sm.tile([NB, 2], F32)
            t1 = sm.tile([NB, 1], F32)
            nc.vector.tensor_scalar_mul(t1[:], gxy[:, 1:2], float(W * A))
            t2 = sm.tile([NB, 1], F32)
            nc.vector.tensor_scalar_mul(t2[:], gxy[:, 0:1], float(A))
            nc.vector.tensor_add(t1[:], t1[:], t2[:])
            nc.vector.tensor_add(cv[:, 0:1], t1[:], best[:])
            offf = sm.tile([NB, 1], F32)
            nc.vector.tensor_scalar(
                offf[:], cv[:, 0:1], BASE, None, op0=mybir.AluOpType.add
            )
            offi = sm.tile([NB, 1], I32)
            nc.vector.tensor_copy(offi[:], offf[:])

            # gather responsible predictions: one indirect DMA, 96 rows of 5
            g96 = sm.tile([NB, C], F32)
            nc.gpsimd.indirect_dma_start(
                out=g96[:],
                out_offset=None,
                in_=gatherv,
                in_offset=bass.IndirectOffsetOnAxis(ap=offi[:], axis=0),
            )

            # validity: any coord nonzero
            vmax = sm.tile([NB, 1], F32)
            nc.vector.tensor_reduce(
                vmax[:], bb[:], axis=mybir.AxisListType.X,
                op=mybir.AluOpType.max, apply_absolute_value=True,
            )
            nc.vector.tensor_scalar(
                cv[:, 1:2], vmax[:], 0.0, None, op0=mybir.AluOpType.is_gt
            )

            # dedup in [4, 24] layout: box i dies if a later valid box j
            # (same image) hits the same cell.  cv hops to 4 partitions and
            # the verdict hops back, both on the scalar queue.
            cvq = sm.tile([BL, NBOX * 2], F32)
            nc.scalar.dma_start(out=cvq[:], in_=cv[:])
            cellq = cvq[:].rearrange("p (i c) -> p i c", c=2)[:, :, 0]
            validq = cvq[:].rearrange("p (i c) -> p i c", c=2)[:, :, 1]
            p3 = lambda ap: ap.rearrange("p (i j) -> p i j", j=NBOX)
            eqp = sm.tile([BL, NBOX * NBOX], F32)
            nc.vector.tensor_tensor(
                out=p3(eqp[:]),
                in0=cellq.to_broadcast([BL, NBOX, NBOX]),
                in1=cellq[:, None, :].broadcast_to([BL, NBOX, NBOX]),
                op=mybir.AluOpType.is_equal,
            )
            nc.vector.tensor_mul(eqp[:], eqp[:], jgt[:])
            nc.vector.tensor_tensor(
                out=p3(eqp[:]), in0=p3(eqp[:]),
                in1=validq[:, None, :].broadcast_to([BL, NBOX, NBOX]),
                op=mybir.AluOpType.mult,
            )
            deadq = sm.tile([BL, NBOX], F32)
            nc.vector.tensor_reduce(
                deadq[:], p3(eqp[:]), axis=mybir.AxisListType.X,
                op=mybir.AluOpType.max,
            )
            dead96 = sm.tile([NB, 1], F32)
            nc.scalar.dma_start(out=dead96[:], in_=deadq[:])
            live = sm.tile([NB, 1], F32)
            nc.vector.tensor_mul(live[:], cv[:, 1:2], dead96[:])
            nc.vector.tensor_sub(live[:], cv[:, 1:2], live[:])

            # targets T = [tx, ty, tw, th] on 96 partitions
            eqb = sm.tile([NB, 9], F32)
            nc.vector.tensor_tensor(
                out=a3(eqb[:]), in0=a3(IOTA),
                in1=best[:].to_broadcast([NB, 1, 9]),
                op=mybir.AluOpType.is_equal,
            )
            selt = sm.tile([NB, 9], F32)
            T96 = sm.tile([NB, 4], F32)
            nc.vector.tensor_sub(T96[:, 0:2], sxy[:], gxy[:])
            nc.vector.tensor_mul(selt[:], eqb[:], RAW)
            rawsel = sm.tile([NB, 1], F32)
            nc.vector.tensor_reduce(
                rawsel[:], a3(selt[:]), axis=mybir.AxisListType.X,
                op=mybir.AluOpType.add,
            )
            nc.vector.tensor_mul(selt[:], eqb[:], RAH)
            rahsel = sm.tile([NB, 1], F32)
            nc.vector.tensor_reduce(
                rahsel[:], a3(selt[:]), axis=mybir.AxisListType.X,
                op=mybir.AluOpType.add,
            )
            nc.vector.tensor_mul(T96[:, 2:3], wv, rawsel[:])
            nc.vector.tensor_mul(T96[:, 3:4], hv, rahsel[:])
            nc.scalar.activation(
                T96[:, 2:4], T96[:, 2:4], mybir.ActivationFunctionType.Ln,
                bias=cst[:, C_EPS:C_EPS + 1],
            )

            # gathered-cell softplus terms and coord residual
            spn = sm.tile([NB, 1], F32)
            spp = sm.tile([NB, 1], F32)
            nc.scalar.activation(
                spn[:], g96[:, 4:5], mybir.ActivationFunctionType.Exp, scale=-1.0
            )
            nc.scalar.activation(
                spn[:], spn[:], mybir.ActivationFunctionType.Ln, bias=1.0
            )
            nc.scalar.activation(
                spp[:], g96[:, 4:5], mybir.ActivationFunctionType.Exp
            )
            nc.scalar.activation(
                spp[:], spp[:], mybir.ActivationFunctionType.Ln, bias=1.0
            )

            diff = sm.tile([NB, 4], F32)
            nc.vector.tensor_sub(diff[:], g96[:, 0:4], T96[:])
            nc.vector.tensor_mul(diff[:], diff[:], diff[:])
            cb = sm.tile([NB, 1], F32)
            nc.vector.tensor_reduce(
                cb[:], diff[:], axis=mybir.AxisListType.X, op=mybir.AluOpType.add
            )

            # partials matrix [96, 4]: cols = 0.5*sub, obj, 5*coord, npos
            rhsm = sm.tile([NB, 4], F32)
            nc.vector.tensor_mul(rhsm[:, 0:1], spp[:], live[:])
            nc.vector.tensor_scalar_mul(rhsm[:, 0:1], rhsm[:, 0:1], LAMBDA_NOOBJ)
            nc.vector.tensor_mul(rhsm[:, 1:2], spn[:], live[:])
            nc.vector.tensor_mul(rhsm[:, 2:3], cb[:], live[:])
            nc.vector.tensor_scalar_mul(rhsm[:, 2:3], rhsm[:, 2:3], LAMBDA_COORD)
            nc.vector.tensor_copy(rhsm[:, 3:4], live[:])

            ps1 = pp.tile([1, 4], F32)
            nc.tensor.matmul(
                ps1[:], lhsT=ones[0:NB, :], rhs=rhsm[:], start=True, stop=True
            )
            bx = sm.tile([1, 4], F32)
            nc.vector.tensor_copy(bx[:], ps1[:])

            # ---- AG#1: box partials (early; arms the collective mesh) ------
            ag1_in = dp.tile([1, 4], F32)
            ag1_out = dp.tile([N_CORES, 4], F32, addr_space="Shared")
            nc.scalar.dma_start(out=ag1_in[:], in_=bx[:])
            nc.gpsimd.collective_compute(
                "AllGather",
                mybir.AluOpType.bypass,
                replica_groups=[list(range(N_CORES))],
                ins=[ag1_in[:].opt()],
                outs=[ag1_out[:].opt()],
            )

            # ---- dense softplus over the obj channel -----------------------
            # softplus(x) = ln(exp(x) + 1); exp and ln share one ACT table set
            accs = sm.tile([P, NCHUNK], F32)
            for i, ch in enumerate(chunks):
                cc = CHUNK_CELLS[i]
                sp = big.tile([P, cc], F32, name=f"sp{i}", tag=f"sp{i}")
                nc.scalar.activation(
                    sp[:], ch[:, 4::5], mybir.ActivationFunctionType.Exp
                )
                nc.scalar.activation(
                    sp[:], sp[:], mybir.ActivationFunctionType.Ln, bias=1.0,
                    accum_out=accs[:, i : i + 1],
                )

            ps2 = pp.tile([1, NCHUNK], F32)
            nc.tensor.matmul(ps2[:], lhsT=ones[:], rhs=accs[:], start=True, stop=True)
            ds = sm.tile([1, 1], F32)
            nc.vector.tensor_reduce(
                ds[:], ps2[:], axis=mybir.AxisListType.X, op=mybir.AluOpType.add
            )

            # ---- AG#2: the 4-byte dense sum (mesh already warm) ------------
            ag2_in = dp.tile([1, 1], F32)
            ag2_out = dp.tile([N_CORES, 1], F32, addr_space="Shared")
            nc.scalar.dma_start(out=ag2_in[:], in_=ds[:])
            nc.gpsimd.collective_compute(
                "AllGather",
                mybir.AluOpType.bypass,
                replica_groups=[list(range(N_CORES))],
                ins=[ag2_in[:].opt()],
                outs=[ag2_out[:].opt()],
            )

            # ---- everything that only needs AG#1 (hidden under the dense
            # tail): global box sums, n_pos/n_neg reciprocals, coord & obj ---
            fin4 = sm.tile([N_CORES, 4], F32)
            nc.scalar.dma_start(out=fin4[:], in_=ag1_out[:])
            psB = pp.tile([1, 4], F32)
            nc.tensor.matmul(
                psB[:], lhsT=ones[0:N_CORES, :], rhs=fin4[:], start=True, stop=True
            )
            gsumB = sm.tile([1, 4], F32)
            nc.vector.tensor_copy(gsumB[:], psB[:])
            # gsumB = [0.5*sub, obj, 5*coord, npos]
            den = sm.tile([1, 2], F32)
            nc.vector.tensor_scalar(
                den[:, 0:1], gsumB[:, 3:4], 1.0, None, op0=mybir.AluOpType.max
            )
            nc.vector.tensor_scalar(
                den[:, 1:2], gsumB[:, 3:4], -1.0, float(TOT_CELLS),
                op0=mybir.AluOpType.mult, op1=mybir.AluOpType.add,
            )
            rec = sm.tile([1, 2], F32)
            nc.vector.reciprocal(rec[:], den[:])
            nc.vector.tensor_tensor(
                out=res[:, 1:2], in0=gsumB[:, 2:3], in1=rec[:, 0:1],
                op=mybir.AluOpType.mult,
            )
            nc.vector.tensor_tensor(
                out=res[:, 2:3], in0=gsumB[:, 1:2], in1=rec[:, 0:1],
                op=mybir.AluOpType.mult,
            )
            nc.vector.tensor_add(res[:, 0:1], res[:, 1:2], res[:, 2:3])

            # ---- final: noobj needs AG#2 -----------------------------------
            fin5 = sm.tile([N_CORES, 1], F32)
            nc.scalar.dma_start(out=fin5[:], in_=ag2_out[:])
            psD = pp.tile([1, 1], F32)
            nc.tensor.matmul(
                psD[:], lhsT=ones[0:N_CORES, :], rhs=fin5[:], start=True, stop=True
            )
            # noobj = (0.5*dense - 0.5*sub) / n_neg
            nc.vector.tensor_scalar(
                res[:, 3:4], psD[:], LAMBDA_NOOBJ, None, op0=mybir.AluOpType.mult
            )
            nc.vector.tensor_sub(res[:, 3:4], res[:, 3:4], gsumB[:, 0:1])
            nc.vector.tensor_tensor(
                out=res[:, 3:4], in0=res[:, 3:4], in1=rec[:, 1:2],
                op=mybir.AluOpType.mult,
            )
            nc.vector.tensor_add(res[:, 0:1], res[:, 0:1], res[:, 3:4])

            nc.sync.dma_start(out=outt[:], in_=res[0:1, 0:5])

    nc.compile()
    return nc


_NC_CACHE = None


def _get_nc():
    global _NC_CACHE
    if _NC_CACHE is None:
        _NC_CACHE = _build_nc()
    return _NC_CACHE


def kernel_with_results(predictions, bboxes, **run_kwargs):
    predictions = np.ascontiguousarray(predictions, dtype=np.float32)
    bboxes = np.ascontiguousarray(bboxes, dtype=np.float32)
    assert predictions.shape == (B, H, W, A, C)
    assert bboxes.shape == (B, NBOX, 4)

    cst = _build_const96()
    jgt = _build_jgt()
    in_maps = []
    for c in range(N_CORES):
        shard_p = predictions[c * BL : (c + 1) * BL].reshape(-1)
        shard_b = bboxes[c * BL : (c + 1) * BL].reshape(NB, 4)
        in_maps.append({"pred": shard_p, "bb": shard_b, "cst": cst, "jgt": jgt})

    nc = _get_nc()
    res = run_bass_kernel_spmd(nc, in_maps, core_ids=list(range(N_CORES)), **run_kwargs)
    out = np.asarray(res.results[0]["out"], dtype=np.float32).reshape(5)
    return out, res


def kernel(predictions, bboxes):
    out, _ = kernel_with_results(predictions, bboxes)
    return out
